# revision 19
# baseline (speedup 1.0000x reference)
"""Trainium2 Bass kernel for EnhancedLocalAttentionWithGQA.

Problem (hardcoded): B=2, L=4096, C=2048, H=16 heads, D=128, G=2 kv groups,
window W=256 with stride 128 (50% overlap).

Key observation: the reference computes NW=31 overlapping windows but the
final output slice [:, :L] keeps only windows 0..15 (16 windows x 256 rows
= 4096 rows).  Window n's output rows [n*256,(n+1)*256) come from queries /
keys / values at input positions [n*128, n*128+256).  So only x positions
0..2175 feed QKV, and each window is an independent 256x256 attention.

Sharding (8 cores): core c -> batch b=c//4, quarter p=c%4, i.e. 4 windows
(global windows 4p..4p+3), input positions [512p, 512p+640), output rows
[1024p, 1024p+1024) of batch b.  No collectives; host concatenates rows.

Per-core pipeline (bf16 matmuls, fp32 PSUM), engineered so the PE never
waits:
  1. V-projection runs first, kc-outer, so it consumes x chunks as their
     DMAs land (x arrival is the startup bottleneck; lhsT=x chunks).
  2. K-projection, then Q per head (feat-major Q^T resident per head pair).
  3. Attention pairs (2 heads of one kv group, N=512 matmuls) interleaved
     1:1 with out-projection blocks of the PREVIOUS window, so softmax's
     ACT exp / DVE recip+mul hide entirely under out-proj matmul streams.
     Softmax sum uses an all-ones 128x128 lhsT so the colsum comes out of
     PSUM already broadcast across partitions (no ACT copy, no extra
     broadcast matmul).

All weights are host-pretiled so every DMA is a large contiguous transfer.
"""

import numpy as np
import ml_dtypes

import concourse.bacc as bacc
import concourse.tile as tile
from concourse import mybir
from concourse.bass_utils import run_bass_kernel_spmd

F32 = mybir.dt.float32
BF16 = mybir.dt.bfloat16

B = 2
L = 4096
C = 2048          # embed dim
H = 16            # heads
G = 2             # kv groups
D = 128           # head dim
KV = G * D        # 256
NWL = 4           # windows per core
S = NWL * 128 + 128   # 640 input positions per core
OUT_ROWS = NWL * 256  # 1024 output rows per core
KC = C // 128     # 16 contraction chunks
NT = 4            # out-proj 512-col tiles
SC_OUT = OUT_ROWS // 128
SCALE = 1.0 / float(np.sqrt(D))
N_CORES = 8

_CACHE = {}


def _build():
    nc = bacc.Bacc(None, target_bir_lowering=False)

    # host-pretiled layouts (see kernel() for the numpy side)
    xT_d = nc.dram_tensor("xT", [128, KC, S], BF16, kind="ExternalInput")
    wq_d = nc.dram_tensor("Wq", [H, 128, KC, 128], BF16, kind="ExternalInput")
    wk_d = nc.dram_tensor("Wk", [G, 128, KC, 128], BF16, kind="ExternalInput")
    wv_d = nc.dram_tensor("Wv", [128, KC, KV], BF16, kind="ExternalInput")
    wo_d = nc.dram_tensor("Wo", [NT, 128, KC, 512], BF16, kind="ExternalInput")
    bq_d = nc.dram_tensor("bq", [C], F32, kind="ExternalInput")
    bk_d = nc.dram_tensor("bk", [KV], F32, kind="ExternalInput")
    bv_d = nc.dram_tensor("bv", [KV], F32, kind="ExternalInput")
    bo_d = nc.dram_tensor("bo", [C], F32, kind="ExternalInput")
    out_d = nc.dram_tensor("out", [NT, SC_OUT, 128, 512], F32,
                           kind="ExternalOutput")

    with tile.TileContext(nc) as tc:
        with (
            tc.tile_pool(name="res", bufs=1) as res,
            tc.tile_pool(name="wqs", bufs=3) as wqs,
            tc.tile_pool(name="pts", bufs=3) as pts,
            tc.tile_pool(name="osb", bufs=3) as osbp,
            tc.tile_pool(name="norm", bufs=3) as norm,
        ):
            # ---------- resident loads (both queues, consumption order) ---
            # The early DMA rate is ~150GB/s per queue but queues add up,
            # so the startup-critical tensors (wv, x, wk, first wq) are
            # striped across the sync AND gpsimd queues: x even chunks on
            # sync, odd on gpsimd.  V-proj consumes x chunks as lhsT the
            # moment they land; then wk (K-proj), per-head wq, resident Wo.
            wv_t = res.tile([128, KC, KV], BF16, tag="wv", name="wv")
            xtq = [res.tile([128, 2, S], BF16, tag=f"xt{j}", name=f"xt{j}")
                   for j in range(8)]
            bq_sb = res.tile([128, H], F32, tag="bq", name="bq")
            nc.gpsimd.dma_start(out=bq_sb, in_=bq_d[:].rearrange("(h p) -> p h", p=128))
            bk_sb = res.tile([128, G], F32, tag="bk", name="bk")
            nc.gpsimd.dma_start(out=bk_sb, in_=bk_d[:].rearrange("(g p) -> p g", p=128))
            bv_bc = res.tile([128, KV], F32, tag="bvbc", name="bvbc")
            nc.gpsimd.dma_start(out=bv_bc,
                              in_=bv_d[:].unsqueeze(0).to_broadcast((128, KV)))

            kw = [res.tile([128, KC, 128], BF16, tag=f"kw{g}", name=f"kw{g}")
                  for g in range(G)]
            wq0s = [res.tile([128, 4, 128], BF16, tag=f"wq0{j}", name=f"wq0{j}")
                    for j in range(4)]
            nc.sync.dma_start(out=wv_t[:, 0:4, :], in_=wv_d[:, 0:4, :])
            nc.sync.dma_start(out=xtq[0], in_=xT_d[:, 0:2, :])
            nc.gpsimd.dma_start(out=xtq[1], in_=xT_d[:, 2:4, :])
            nc.sync.dma_start(out=wv_t[:, 4:8, :], in_=wv_d[:, 4:8, :])
            nc.sync.dma_start(out=xtq[2], in_=xT_d[:, 4:6, :])
            nc.gpsimd.dma_start(out=xtq[3], in_=xT_d[:, 6:8, :])
            nc.sync.dma_start(out=wv_t[:, 8:12, :], in_=wv_d[:, 8:12, :])
            nc.sync.dma_start(out=xtq[4], in_=xT_d[:, 8:10, :])
            nc.gpsimd.dma_start(out=kw[0], in_=wk_d[0])
            nc.gpsimd.dma_start(out=xtq[5], in_=xT_d[:, 10:12, :])
            nc.sync.dma_start(out=wv_t[:, 12:16, :], in_=wv_d[:, 12:16, :])
            nc.sync.dma_start(out=xtq[6], in_=xT_d[:, 12:14, :])
            nc.gpsimd.dma_start(out=xtq[7], in_=xT_d[:, 14:16, :])
            nc.gpsimd.dma_start(out=kw[1], in_=wk_d[1])
            for j in range(4):
                nc.gpsimd.dma_start(out=wq0s[j], in_=wq_d[0][:, j * 4:(j + 1) * 4, :])

            def xts(kc):
                return xtq[kc // 2][:, kc % 2, :]

            # needed only from the first out-proj block (~100us in)
            bo_bc = res.tile([128, C], F32, tag="bobc", name="bobc")
            nc.gpsimd.dma_start(out=bo_bc,
                              in_=bo_d[:].unsqueeze(0).to_broadcast((128, C)))

            ones128 = res.tile([128, 128], BF16, tag="ones", name="ones")
            nc.vector.memset(ones128, 1.0)
            zcol = res.tile([128, 1], F32, tag="zcol", name="zcol")
            nc.vector.memset(zcol, 0.0)
            # dummy exp: preload the ACT exp table set while the PE is
            # still waiting on the x DMA (the real first exp would other-
            # wise eat the ~2.7us table load mid-pipeline).
            dummy = res.tile([128, 1], F32, tag="dummy", name="dummy")
            nc.scalar.activation(dummy, zcol,
                                 mybir.ActivationFunctionType.Exp)

            # paired Q storage: qp[g*4+j] holds heads (g+4j, g+4j+2)
            qp = [res.tile([128, 2, S], BF16, tag=f"qp{i}", name=f"qp{i}")
                  for i in range(8)]

            def q_slot(h):
                g, k = h % G, h // G
                return qp[g * 4 + k // 2][:, k % 2, :]
            kt = [res.tile([128, S], BF16, tag=f"kt{g}", name=f"kt{g}")
                  for g in range(G)]
            vt = [res.tile([128, KV], BF16, tag=f"vt{sc}", name=f"vt{sc}")
                  for sc in range(S // 128)]
            # O^T per window (separate tiles so out-proj reads of window
            # w-1 never alias the concurrent writes of window w)
            otw = [res.tile([128, H, 256], BF16, tag=f"ot{w}", name=f"ot{w}")
                   for w in range(NWL)]

            NA, NB = 320, 320  # free split of S=640 (psum bank = 512 f32)

            # ---------- V projection ----------
            with tc.tile_pool(name="psA", bufs=1, space="PSUM") as psA:
                # HAM warm-up: the PE clock-gate defaults to 4/8 (1.2GHz)
                # and needs ~3.4us of sustained matmul activity to open.
                # The PE idles waiting on the x DMA anyway, so burn that
                # window on dummy matmuls and enter V/K projections warm.
                warm = psA.tile([128, 128], F32, tag="warm", name="warm")
                for _ in range(64):
                    nc.tensor.matmul(warm, lhsT=ones128, rhs=ones128,
                                     start=True, stop=True)
                # V + head-0 Q, kc-outer: each x chunk enables matmuls the
                # moment it lands, so the PE tracks the x DMA instead of
                # idling behind it.  Accumulation order follows expected
                # DMA arrival (sync evens lead; the gpsimd SWDGE queue
                # starts ~6us later).  PSUM: warm 1 + pv 5 + pa/pb 2 = 8.
                pvs = [psA.tile([128, KV], F32, tag=f"vp{sc}", name=f"vp{sc}")
                       for sc in range(S // 128)]
                vorder = [0, 1, 4, 5, 2, 3, 8, 9, 6, 7, 12, 13, 10, 11, 14, 15]
                for idx, kc in enumerate(vorder):
                    for sc in range(S // 128):
                        nc.tensor.matmul(
                            pvs[sc], lhsT=xts(kc)[:, sc * 128:(sc + 1) * 128],
                            rhs=wv_t[:, kc, :],
                            start=(idx == 0), stop=(idx == KC - 1))
                for sc in range(S // 128):
                    nc.vector.tensor_add(vt[sc], pvs[sc], bv_bc)

            # resident Wo, queued behind the wq head stream on purpose:
            # needed only from the first out-proj block (~100us in).
            wo_all = [res.tile([128, KC, 512], BF16, tag=f"wo{nt}",
                               name=f"wo{nt}") for nt in range(NT)]

            # ---------- K/Q projections + attention + out-projection ------
            # pair i = (window w, kv group g, head pair j): heads g+4j and
            # g+4j+2, query cols w*128..w*128+256.  Window 0's pairs are
            # injected into the Q-head stream (their ACT/DVE work hides
            # under the projection matmuls); pairs of windows 1..3 are
            # interleaved 1:1 with out-proj blocks of window w-1, so the
            # softmax's ACT/DVE work never gates the PE.
            pairs = [(w, g, j) for w in range(NWL)
                     for g in range(G) for j in range(4)]
            NP = len(pairs)
            pt_tiles = {}

            with (
                tc.tile_pool(name="psB", bufs=1, space="PSUM") as psB,
                tc.tile_pool(name="psS", bufs=1, space="PSUM") as psS,
                tc.tile_pool(name="psO", bufs=1, space="PSUM") as psO,
                tc.tile_pool(name="psC", bufs=1, space="PSUM") as psC,
                tc.tile_pool(name="psP", bufs=2, space="PSUM") as psP,
            ):
                for g in range(G):
                    pa = psB.tile([128, NA], F32, tag="qa", name="qa")
                    pb = psB.tile([128, NB], F32, tag="qb", name="qb")
                    for kc in range(KC):
                        nc.tensor.matmul(pa, lhsT=kw[g][:, kc, :],
                                         rhs=xts(kc)[:, 0:NA],
                                         start=(kc == 0), stop=(kc == KC - 1))
                    for kc in range(KC):
                        nc.tensor.matmul(pb, lhsT=kw[g][:, kc, :],
                                         rhs=xts(kc)[:, NA:S],
                                         start=(kc == 0), stop=(kc == KC - 1))
                    nc.scalar.activation(kt[g][:, 0:NA], pa,
                                         mybir.ActivationFunctionType.Identity,
                                         bias=bk_sb[:, g:g + 1])
                    nc.scalar.activation(kt[g][:, NA:S], pb,
                                         mybir.ActivationFunctionType.Identity,
                                         bias=bk_sb[:, g:g + 1])
                def pair_qk(i):
                    w, g, j = pairs[i]
                    q0 = w * 128
                    qpt = qp[g * 4 + j]
                    st = psS.tile([128, 2, 512], F32, tag="st", name="st")
                    for kc in range(2):
                        nc.tensor.matmul(
                            st[:, kc, :],
                            lhsT=kt[g][:, q0 + kc * 128:q0 + (kc + 1) * 128],
                            rhs=qpt[:, :, q0:q0 + 256],
                            start=True, stop=True)
                    pt = pts.tile([128, 2, 512], BF16, tag="pt", name="pt")
                    nc.scalar.activation(pt, st,
                                         mybir.ActivationFunctionType.Exp,
                                         scale=SCALE)
                    pt_tiles[i] = pt

                def pair_av(i):
                    w, g, j = pairs[i]
                    h0, h1 = g + 4 * j, g + 4 * j + 2
                    pt = pt_tiles.pop(i)
                    ob = psO.tile([128, 512], F32, tag="ob", name="ob")
                    for kc in range(2):
                        nc.tensor.matmul(ob,
                                         lhsT=vt[w + kc][:, g * 128:(g + 1) * 128],
                                         rhs=pt[:, kc, :],
                                         start=(kc == 0), stop=(kc == 1))
                    # all-ones lhsT: every PSUM partition gets the colsum,
                    # i.e. the softmax denominator arrives pre-broadcast.
                    bc = psC.tile([128, 512], F32, tag="bc", name="bc")
                    for kc in range(2):
                        nc.tensor.matmul(bc, lhsT=ones128, rhs=pt[:, kc, :],
                                         start=(kc == 0), stop=(kc == 1))
                    bcr = norm.tile([128, 512], F32, tag="bcr", name="bcr")
                    nc.vector.reciprocal_approx_fast(out=bcr, in_=bc)
                    nc.vector.tensor_mul(otw[w][:, h0, :], ob[:, 0:256],
                                         bcr[:, 0:256])
                    nc.vector.tensor_mul(otw[w][:, h1, :], ob[:, 256:512],
                                         bcr[:, 256:512])

                def outproj_block(w_src, b, last=False):
                    nt, half = b % NT, b // NT
                    sc = 2 * w_src + half
                    po = psP.tile([128, 512], F32, tag="op", name="op")
                    for fc in range(KC):
                        nc.tensor.matmul(
                            po,
                            lhsT=otw[w_src][:, fc, half * 128:(half + 1) * 128],
                            rhs=wo_all[nt][:, fc, :],
                            start=(fc == 0), stop=(fc == KC - 1))
                    ob_sb = osbp.tile([128, 512], F32, tag="osb", name="osb")
                    if last:
                        # split the final block so its DVE add and DMA
                        # pipeline instead of serializing the kernel tail
                        for hx in range(4):
                            cs = slice(hx * 128, (hx + 1) * 128)
                            nc.vector.tensor_add(ob_sb[:, cs], po[:, cs],
                                                 bo_bc[:, nt * 512 + hx * 128:
                                                       nt * 512 + (hx + 1) * 128])
                            nc.sync.dma_start(
                                out=out_d[nt, sc, :, cs], in_=ob_sb[:, cs])
                    else:
                        nc.vector.tensor_add(ob_sb, po,
                                             bo_bc[:, nt * 512:(nt + 1) * 512])
                        nc.sync.dma_start(out=out_d[nt, sc], in_=ob_sb)

                def q_head(h):
                    if h > 0:
                        wq_t = wqs.tile([128, KC, 128], BF16, tag="wq", name="wq")
                        nc.sync.dma_start(out=wq_t, in_=wq_d[h])

                    def wql(kc, h=h, wq_t=(None if h == 0 else wq_t)):
                        if h == 0:
                            return wq0s[kc // 4][:, kc % 4, :]
                        return wq_t[:, kc, :]
                    pa = psB.tile([128, NA], F32, tag="qa", name="qa")
                    pb = psB.tile([128, NB], F32, tag="qb", name="qb")
                    for kc in range(KC):
                        nc.tensor.matmul(pa, lhsT=wql(kc),
                                         rhs=xts(kc)[:, 0:NA],
                                         start=(kc == 0), stop=(kc == KC - 1))
                    for kc in range(KC):
                        nc.tensor.matmul(pb, lhsT=wql(kc),
                                         rhs=xts(kc)[:, NA:S],
                                         start=(kc == 0), stop=(kc == KC - 1))
                    nc.scalar.activation(q_slot(h)[:, 0:NA], pa,
                                         mybir.ActivationFunctionType.Identity,
                                         bias=bq_sb[:, h:h + 1])
                    nc.scalar.activation(q_slot(h)[:, NA:S], pb,
                                         mybir.ActivationFunctionType.Identity,
                                         bias=bq_sb[:, h:h + 1])

                # Q heads in pair order, window-0 pair p injected as soon
                # as its two heads exist (pairs[p] = (0, p//4, p%4) covers
                # heads p//4 + 4*(p%4) and +2, i.e. exactly one hp pair).
                for p in range(8):
                    g, j = p // 4, p % 4
                    q_head(g + 4 * j)
                    q_head(g + 4 * j + 2)
                    pair_qk(p)
                    if p > 0:
                        pair_av(p - 1)
                pair_av(7)
                for nt in range(NT):
                    nc.sync.dma_start(out=wo_all[nt], in_=wo_d[nt])

                pair_qk(8)
                for i in range(8, NP):
                    pair_av(i)
                    if i + 1 < NP:
                        pair_qk(i + 1)
                    outproj_block(pairs[i][0] - 1, i % 8)
                for b in range(8):
                    outproj_block(NWL - 1, b, last=(b == 7))

    nc.compile()
    return nc


def _get_nc():
    if "nc" not in _CACHE:
        _CACHE["nc"] = _build()
    return _CACHE["nc"]


def _prep_weights(Wq, bq, Wk, bk, Wv, bv, Wo, bo):
    bf16 = ml_dtypes.bfloat16
    f32 = lambda a: np.ascontiguousarray(np.asarray(a, dtype=np.float32))
    wq = np.asarray(Wq, np.float32).reshape(KC, 128, H, 128)
    wq = np.ascontiguousarray(wq.transpose(2, 1, 0, 3)).astype(bf16)  # (H,p,kc,f)
    wk = np.asarray(Wk, np.float32).reshape(KC, 128, G, 128)
    wk = np.ascontiguousarray(wk.transpose(2, 1, 0, 3)).astype(bf16)  # (G,p,kc,f)
    wv = np.asarray(Wv, np.float32).reshape(KC, 128, KV)
    wv = np.ascontiguousarray(wv.transpose(1, 0, 2)).astype(bf16)     # (p,kc,f)
    wo = np.asarray(Wo, np.float32).reshape(KC, 128, NT, 512)
    wo = np.ascontiguousarray(wo.transpose(2, 1, 0, 3)).astype(bf16)  # (NT,p,kc,f)
    return {
        "Wq": wq, "Wk": wk, "Wv": wv, "Wo": wo,
        "bq": f32(bq), "bk": f32(bk), "bv": f32(bv), "bo": f32(bo),
    }


def _prep_in_maps(x, weights):
    bf16 = ml_dtypes.bfloat16
    in_maps = []
    for c in range(N_CORES):
        b, p = divmod(c, 4)
        xs = np.asarray(x[b, 512 * p:512 * p + S, :], np.float32)
        xT = np.ascontiguousarray(xs.T.reshape(KC, 128, S).transpose(1, 0, 2))
        in_maps.append(dict(weights, xT=xT.astype(bf16)))
    return in_maps


def kernel(x, Wq, bq, Wk, bk, Wv, bv, Wo, bo, **_):
    x = np.asarray(x, dtype=np.float32)
    weights = _prep_weights(Wq, bq, Wk, bk, Wv, bv, Wo, bo)
    in_maps = _prep_in_maps(x, weights)

    nc = _get_nc()
    res = run_bass_kernel_spmd(nc, in_maps, core_ids=list(range(N_CORES)))

    out = np.empty((B, L, C), dtype=np.float32)
    for c in range(N_CORES):
        b, p = divmod(c, 4)
        blk = res.results[c]["out"]  # (NT, SC_OUT, 128, 512)
        rows = blk.transpose(1, 2, 0, 3).reshape(OUT_ROWS, C)
        out[b, 1024 * p:1024 * p + OUT_ROWS, :] = rows
    return out


# revision 22
# speedup vs baseline: 1.0108x; 1.0108x over previous
"""Trainium2 Bass kernel for EnhancedLocalAttentionWithGQA.

Problem (hardcoded): B=2, L=4096, C=2048, H=16 heads, D=128, G=2 kv groups,
window W=256 with stride 128 (50% overlap).

Key observation: the reference computes NW=31 overlapping windows but the
final output slice [:, :L] keeps only windows 0..15 (16 windows x 256 rows
= 4096 rows).  Window n's output rows [n*256,(n+1)*256) come from queries /
keys / values at input positions [n*128, n*128+256).  So only x positions
0..2175 feed QKV, and each window is an independent 256x256 attention.

Sharding (8 cores): core c -> batch b=c//4, quarter p=c%4, i.e. 4 windows
(global windows 4p..4p+3), input positions [512p, 512p+640), output rows
[1024p, 1024p+1024) of batch b.  No collectives; host concatenates rows.

Per-core pipeline (bf16 matmuls, fp32 PSUM), engineered so the PE never
waits:
  1. V-projection runs first, kc-outer, so it consumes x chunks as their
     DMAs land (x arrival is the startup bottleneck; lhsT=x chunks).
  2. K-projection, then Q per head (feat-major Q^T resident per head pair).
  3. Attention pairs (2 heads of one kv group, N=512 matmuls) interleaved
     1:1 with out-projection blocks of the PREVIOUS window, so softmax's
     ACT exp / DVE recip+mul hide entirely under out-proj matmul streams.
     Softmax sum uses an all-ones 128x128 lhsT so the colsum comes out of
     PSUM already broadcast across partitions (no ACT copy, no extra
     broadcast matmul).

All weights are host-pretiled so every DMA is a large contiguous transfer.
"""

import numpy as np
import ml_dtypes

import concourse.bacc as bacc
import concourse.tile as tile
from concourse import mybir
from concourse.bass_utils import run_bass_kernel_spmd

F32 = mybir.dt.float32
BF16 = mybir.dt.bfloat16

B = 2
L = 4096
C = 2048          # embed dim
H = 16            # heads
G = 2             # kv groups
D = 128           # head dim
KV = G * D        # 256
NWL = 4           # windows per core
S = NWL * 128 + 128   # 640 input positions per core
OUT_ROWS = NWL * 256  # 1024 output rows per core
KC = C // 128     # 16 contraction chunks
NT = 4            # out-proj 512-col tiles
SC_OUT = OUT_ROWS // 128
SCALE = 1.0 / float(np.sqrt(D))
N_CORES = 8

_CACHE = {}


def _build():
    nc = bacc.Bacc(None, target_bir_lowering=False)

    # host-pretiled layouts (see kernel() for the numpy side)
    xT_d = nc.dram_tensor("xT", [128, KC, S], BF16, kind="ExternalInput")
    wq_d = nc.dram_tensor("Wq", [H, 128, KC, 128], BF16, kind="ExternalInput")
    wk_d = nc.dram_tensor("Wk", [G, 128, KC, 128], BF16, kind="ExternalInput")
    wv_d = nc.dram_tensor("Wv", [128, KC, KV], BF16, kind="ExternalInput")
    wo_d = nc.dram_tensor("Wo", [NT, 128, KC, 512], BF16, kind="ExternalInput")
    bq_d = nc.dram_tensor("bq", [C], F32, kind="ExternalInput")
    bk_d = nc.dram_tensor("bk", [KV], F32, kind="ExternalInput")
    bv_d = nc.dram_tensor("bv", [KV], F32, kind="ExternalInput")
    bo_d = nc.dram_tensor("bo", [C], F32, kind="ExternalInput")
    out_d = nc.dram_tensor("out", [NT, SC_OUT, 128, 512], F32,
                           kind="ExternalOutput")

    with tile.TileContext(nc) as tc:
        with (
            tc.tile_pool(name="res", bufs=1) as res,
            tc.tile_pool(name="wqs", bufs=3) as wqs,
            tc.tile_pool(name="pts", bufs=3) as pts,
            tc.tile_pool(name="osb", bufs=3) as osbp,
            tc.tile_pool(name="norm", bufs=3) as norm,
        ):
            # ---------- resident loads (both queues, consumption order) ---
            # The early DMA rate is ~150GB/s per queue but queues add up,
            # so the startup-critical tensors (wv, x, wk, first wq) are
            # striped across the sync AND gpsimd queues: x even chunks on
            # sync, odd on gpsimd.  V-proj consumes x chunks as lhsT the
            # moment they land; then wk (K-proj), per-head wq, resident Wo.
            wv_t = res.tile([128, KC, KV], BF16, tag="wv", name="wv")
            xtq = [res.tile([128, 2, S], BF16, tag=f"xt{j}", name=f"xt{j}")
                   for j in range(8)]
            bq_sb = res.tile([128, H], F32, tag="bq", name="bq")
            nc.gpsimd.dma_start(out=bq_sb, in_=bq_d[:].rearrange("(h p) -> p h", p=128))
            bk_sb = res.tile([128, G], F32, tag="bk", name="bk")
            nc.gpsimd.dma_start(out=bk_sb, in_=bk_d[:].rearrange("(g p) -> p g", p=128))
            bv_bc = res.tile([128, KV], F32, tag="bvbc", name="bvbc")
            nc.gpsimd.dma_start(out=bv_bc,
                              in_=bv_d[:].unsqueeze(0).to_broadcast((128, KV)))

            kw = [res.tile([128, KC, 128], BF16, tag=f"kw{g}", name=f"kw{g}")
                  for g in range(G)]
            wq0s = [res.tile([128, 4, 128], BF16, tag=f"wq0{j}", name=f"wq0{j}")
                    for j in range(4)]
            # x rides mostly on the sync queue (the gpsimd SWDGE queue has
            # multi-us startup jitter); only the three latest-consumed x
            # chunks go to gpsimd, followed by wk.
            nc.sync.dma_start(out=wv_t[:, 0:4, :], in_=wv_d[:, 0:4, :])
            nc.sync.dma_start(out=xtq[0], in_=xT_d[:, 0:2, :])
            nc.gpsimd.dma_start(out=xtq[3], in_=xT_d[:, 6:8, :])
            nc.sync.dma_start(out=xtq[1], in_=xT_d[:, 2:4, :])
            nc.sync.dma_start(out=wv_t[:, 4:8, :], in_=wv_d[:, 4:8, :])
            nc.sync.dma_start(out=xtq[2], in_=xT_d[:, 4:6, :])
            nc.gpsimd.dma_start(out=xtq[5], in_=xT_d[:, 10:12, :])
            nc.sync.dma_start(out=wv_t[:, 8:12, :], in_=wv_d[:, 8:12, :])
            nc.sync.dma_start(out=xtq[4], in_=xT_d[:, 8:10, :])
            nc.gpsimd.dma_start(out=xtq[7], in_=xT_d[:, 14:16, :])
            nc.sync.dma_start(out=wv_t[:, 12:16, :], in_=wv_d[:, 12:16, :])
            nc.sync.dma_start(out=xtq[6], in_=xT_d[:, 12:14, :])
            nc.gpsimd.dma_start(out=kw[0], in_=wk_d[0])
            nc.gpsimd.dma_start(out=kw[1], in_=wk_d[1])
            for j in range(4):
                nc.sync.dma_start(out=wq0s[j], in_=wq_d[0][:, j * 4:(j + 1) * 4, :])

            def xts(kc):
                return xtq[kc // 2][:, kc % 2, :]

            # needed only from the first out-proj block (~100us in)
            bo_bc = res.tile([128, C], F32, tag="bobc", name="bobc")
            nc.gpsimd.dma_start(out=bo_bc,
                              in_=bo_d[:].unsqueeze(0).to_broadcast((128, C)))

            ones128 = res.tile([128, 128], BF16, tag="ones", name="ones")
            nc.vector.memset(ones128, 1.0)
            zcol = res.tile([128, 1], F32, tag="zcol", name="zcol")
            nc.vector.memset(zcol, 0.0)
            # dummy exp: preload the ACT exp table set while the PE is
            # still waiting on the x DMA (the real first exp would other-
            # wise eat the ~2.7us table load mid-pipeline).
            dummy = res.tile([128, 1], F32, tag="dummy", name="dummy")
            nc.scalar.activation(dummy, zcol,
                                 mybir.ActivationFunctionType.Exp)

            # paired Q storage: qp[g*4+j] holds heads (g+4j, g+4j+2)
            qp = [res.tile([128, 2, S], BF16, tag=f"qp{i}", name=f"qp{i}")
                  for i in range(8)]

            def q_slot(h):
                g, k = h % G, h // G
                return qp[g * 4 + k // 2][:, k % 2, :]
            kt = [res.tile([128, S], BF16, tag=f"kt{g}", name=f"kt{g}")
                  for g in range(G)]
            vt = [res.tile([128, KV], BF16, tag=f"vt{sc}", name=f"vt{sc}")
                  for sc in range(S // 128)]
            # O^T per window (separate tiles so out-proj reads of window
            # w-1 never alias the concurrent writes of window w)
            otw = [res.tile([128, H, 256], BF16, tag=f"ot{w}", name=f"ot{w}")
                   for w in range(NWL)]

            NA, NB = 320, 320  # free split of S=640 (psum bank = 512 f32)

            # ---------- V projection ----------
            with tc.tile_pool(name="psA", bufs=1, space="PSUM") as psA:
                # HAM warm-up: the PE clock-gate defaults to 4/8 (1.2GHz)
                # and needs ~3.4us of sustained matmul activity to open.
                # The PE idles waiting on the x DMA anyway, so burn that
                # window on dummy matmuls and enter V/K projections warm.
                warm = psA.tile([128, 128], F32, tag="warm", name="warm")
                for _ in range(64):
                    nc.tensor.matmul(warm, lhsT=ones128, rhs=ones128,
                                     start=True, stop=True)
                # V + head-0 Q, kc-outer: each x chunk enables matmuls the
                # moment it lands, so the PE tracks the x DMA instead of
                # idling behind it.  Accumulation order follows expected
                # DMA arrival (sync evens lead; the gpsimd SWDGE queue
                # starts ~6us later).  PSUM: warm 1 + pv 5 + pa/pb 2 = 8.
                pvs = [psA.tile([128, KV], F32, tag=f"vp{sc}", name=f"vp{sc}")
                       for sc in range(S // 128)]
                vorder = [0, 1, 2, 3, 4, 5, 8, 9, 6, 7, 12, 13, 10, 11, 14, 15]
                for idx, kc in enumerate(vorder):
                    for sc in range(S // 128):
                        nc.tensor.matmul(
                            pvs[sc], lhsT=xts(kc)[:, sc * 128:(sc + 1) * 128],
                            rhs=wv_t[:, kc, :],
                            start=(idx == 0), stop=(idx == KC - 1))
                for sc in range(S // 128):
                    nc.vector.tensor_add(vt[sc], pvs[sc], bv_bc)

            # resident Wo, queued behind the wq head stream on purpose:
            # needed only from the first out-proj block (~100us in).
            wo_all = [res.tile([128, KC, 512], BF16, tag=f"wo{nt}",
                               name=f"wo{nt}") for nt in range(NT)]

            # ---------- K/Q projections + attention + out-projection ------
            # pair i = (window w, kv group g, head pair j): heads g+4j and
            # g+4j+2, query cols w*128..w*128+256.  Window 0's pairs are
            # injected into the Q-head stream (their ACT/DVE work hides
            # under the projection matmuls); pairs of windows 1..3 are
            # interleaved 1:1 with out-proj blocks of window w-1, so the
            # softmax's ACT/DVE work never gates the PE.
            pairs = [(w, g, j) for w in range(NWL)
                     for g in range(G) for j in range(4)]
            NP = len(pairs)
            pt_tiles = {}

            with (
                tc.tile_pool(name="psB", bufs=1, space="PSUM") as psB,
                tc.tile_pool(name="psS", bufs=1, space="PSUM") as psS,
                tc.tile_pool(name="psO", bufs=1, space="PSUM") as psO,
                tc.tile_pool(name="psC", bufs=1, space="PSUM") as psC,
                tc.tile_pool(name="psP", bufs=2, space="PSUM") as psP,
            ):
                korder = [0, 1, 2, 3, 4, 5, 8, 9, 6, 7, 12, 13, 10, 11, 14, 15]
                for g in range(G):
                    pa = psB.tile([128, NA], F32, tag="qa", name="qa")
                    pb = psB.tile([128, NB], F32, tag="qb", name="qb")
                    for idx, kc in enumerate(korder):
                        nc.tensor.matmul(pa, lhsT=kw[g][:, kc, :],
                                         rhs=xts(kc)[:, 0:NA],
                                         start=(idx == 0), stop=(idx == KC - 1))
                    for kc in range(KC):
                        nc.tensor.matmul(pb, lhsT=kw[g][:, kc, :],
                                         rhs=xts(kc)[:, NA:S],
                                         start=(kc == 0), stop=(kc == KC - 1))
                    nc.scalar.activation(kt[g][:, 0:NA], pa,
                                         mybir.ActivationFunctionType.Identity,
                                         bias=bk_sb[:, g:g + 1])
                    nc.scalar.activation(kt[g][:, NA:S], pb,
                                         mybir.ActivationFunctionType.Identity,
                                         bias=bk_sb[:, g:g + 1])
                def pair_qk(i):
                    w, g, j = pairs[i]
                    q0 = w * 128
                    qpt = qp[g * 4 + j]
                    st = psS.tile([128, 2, 512], F32, tag="st", name="st")
                    for kc in range(2):
                        nc.tensor.matmul(
                            st[:, kc, :],
                            lhsT=kt[g][:, q0 + kc * 128:q0 + (kc + 1) * 128],
                            rhs=qpt[:, :, q0:q0 + 256],
                            start=True, stop=True)
                    pt = pts.tile([128, 2, 512], BF16, tag="pt", name="pt")
                    nc.scalar.activation(pt, st,
                                         mybir.ActivationFunctionType.Exp,
                                         scale=SCALE)
                    pt_tiles[i] = pt

                def pair_av(i):
                    w, g, j = pairs[i]
                    h0, h1 = g + 4 * j, g + 4 * j + 2
                    pt = pt_tiles.pop(i)
                    ob = psO.tile([128, 512], F32, tag="ob", name="ob")
                    for kc in range(2):
                        nc.tensor.matmul(ob,
                                         lhsT=vt[w + kc][:, g * 128:(g + 1) * 128],
                                         rhs=pt[:, kc, :],
                                         start=(kc == 0), stop=(kc == 1))
                    # all-ones lhsT: every PSUM partition gets the colsum,
                    # i.e. the softmax denominator arrives pre-broadcast.
                    bc = psC.tile([128, 512], F32, tag="bc", name="bc")
                    for kc in range(2):
                        nc.tensor.matmul(bc, lhsT=ones128, rhs=pt[:, kc, :],
                                         start=(kc == 0), stop=(kc == 1))
                    bcr = norm.tile([128, 512], F32, tag="bcr", name="bcr")
                    nc.vector.reciprocal_approx_fast(out=bcr, in_=bc)
                    nc.vector.tensor_mul(otw[w][:, h0, :], ob[:, 0:256],
                                         bcr[:, 0:256])
                    nc.vector.tensor_mul(otw[w][:, h1, :], ob[:, 256:512],
                                         bcr[:, 256:512])

                def outproj_block(w_src, b, last=False):
                    nt, half = b % NT, b // NT
                    sc = 2 * w_src + half
                    po = psP.tile([128, 512], F32, tag="op", name="op")
                    for fc in range(KC):
                        nc.tensor.matmul(
                            po,
                            lhsT=otw[w_src][:, fc, half * 128:(half + 1) * 128],
                            rhs=wo_all[nt][:, fc, :],
                            start=(fc == 0), stop=(fc == KC - 1))
                    ob_sb = osbp.tile([128, 512], F32, tag="osb", name="osb")
                    if last:
                        # split the final block so its DVE add and DMA
                        # pipeline instead of serializing the kernel tail
                        for hx in range(4):
                            cs = slice(hx * 128, (hx + 1) * 128)
                            nc.vector.tensor_add(ob_sb[:, cs], po[:, cs],
                                                 bo_bc[:, nt * 512 + hx * 128:
                                                       nt * 512 + (hx + 1) * 128])
                            nc.sync.dma_start(
                                out=out_d[nt, sc, :, cs], in_=ob_sb[:, cs])
                    else:
                        nc.vector.tensor_add(ob_sb, po,
                                             bo_bc[:, nt * 512:(nt + 1) * 512])
                        nc.sync.dma_start(out=out_d[nt, sc], in_=ob_sb)

                def q_head(h):
                    if h > 0:
                        wq_t = wqs.tile([128, KC, 128], BF16, tag="wq", name="wq")
                        nc.sync.dma_start(out=wq_t, in_=wq_d[h])

                    def wql(kc, h=h, wq_t=(None if h == 0 else wq_t)):
                        if h == 0:
                            return wq0s[kc // 4][:, kc % 4, :]
                        return wq_t[:, kc, :]
                    pa = psB.tile([128, NA], F32, tag="qa", name="qa")
                    pb = psB.tile([128, NB], F32, tag="qb", name="qb")
                    for kc in range(KC):
                        nc.tensor.matmul(pa, lhsT=wql(kc),
                                         rhs=xts(kc)[:, 0:NA],
                                         start=(kc == 0), stop=(kc == KC - 1))
                    for kc in range(KC):
                        nc.tensor.matmul(pb, lhsT=wql(kc),
                                         rhs=xts(kc)[:, NA:S],
                                         start=(kc == 0), stop=(kc == KC - 1))
                    nc.scalar.activation(q_slot(h)[:, 0:NA], pa,
                                         mybir.ActivationFunctionType.Identity,
                                         bias=bq_sb[:, h:h + 1])
                    nc.scalar.activation(q_slot(h)[:, NA:S], pb,
                                         mybir.ActivationFunctionType.Identity,
                                         bias=bq_sb[:, h:h + 1])

                # Q heads in pair order, window-0 pair p injected as soon
                # as its two heads exist (pairs[p] = (0, p//4, p%4) covers
                # heads p//4 + 4*(p%4) and +2, i.e. exactly one hp pair).
                for p in range(8):
                    g, j = p // 4, p % 4
                    q_head(g + 4 * j)
                    q_head(g + 4 * j + 2)
                    pair_qk(p)
                    if p > 0:
                        pair_av(p - 1)
                pair_av(7)
                for nt in range(NT):
                    nc.sync.dma_start(out=wo_all[nt], in_=wo_d[nt])

                pair_qk(8)
                for i in range(8, NP):
                    pair_av(i)
                    if i + 1 < NP:
                        pair_qk(i + 1)
                    outproj_block(pairs[i][0] - 1, i % 8)
                for b in range(8):
                    outproj_block(NWL - 1, b, last=(b == 7))

    nc.compile()
    return nc


def _get_nc():
    if "nc" not in _CACHE:
        _CACHE["nc"] = _build()
    return _CACHE["nc"]


def _prep_weights(Wq, bq, Wk, bk, Wv, bv, Wo, bo):
    bf16 = ml_dtypes.bfloat16
    f32 = lambda a: np.ascontiguousarray(np.asarray(a, dtype=np.float32))
    wq = np.asarray(Wq, np.float32).reshape(KC, 128, H, 128)
    wq = np.ascontiguousarray(wq.transpose(2, 1, 0, 3)).astype(bf16)  # (H,p,kc,f)
    wk = np.asarray(Wk, np.float32).reshape(KC, 128, G, 128)
    wk = np.ascontiguousarray(wk.transpose(2, 1, 0, 3)).astype(bf16)  # (G,p,kc,f)
    wv = np.asarray(Wv, np.float32).reshape(KC, 128, KV)
    wv = np.ascontiguousarray(wv.transpose(1, 0, 2)).astype(bf16)     # (p,kc,f)
    wo = np.asarray(Wo, np.float32).reshape(KC, 128, NT, 512)
    wo = np.ascontiguousarray(wo.transpose(2, 1, 0, 3)).astype(bf16)  # (NT,p,kc,f)
    return {
        "Wq": wq, "Wk": wk, "Wv": wv, "Wo": wo,
        "bq": f32(bq), "bk": f32(bk), "bv": f32(bv), "bo": f32(bo),
    }


def _prep_in_maps(x, weights):
    bf16 = ml_dtypes.bfloat16
    in_maps = []
    for c in range(N_CORES):
        b, p = divmod(c, 4)
        xs = np.asarray(x[b, 512 * p:512 * p + S, :], np.float32)
        xT = np.ascontiguousarray(xs.T.reshape(KC, 128, S).transpose(1, 0, 2))
        in_maps.append(dict(weights, xT=xT.astype(bf16)))
    return in_maps


def kernel(x, Wq, bq, Wk, bk, Wv, bv, Wo, bo, **_):
    x = np.asarray(x, dtype=np.float32)
    weights = _prep_weights(Wq, bq, Wk, bk, Wv, bv, Wo, bo)
    in_maps = _prep_in_maps(x, weights)

    nc = _get_nc()
    res = run_bass_kernel_spmd(nc, in_maps, core_ids=list(range(N_CORES)))

    out = np.empty((B, L, C), dtype=np.float32)
    for c in range(N_CORES):
        b, p = divmod(c, 4)
        blk = res.results[c]["out"]  # (NT, SC_OUT, 128, 512)
        rows = blk.transpose(1, 2, 0, 3).reshape(OUT_ROWS, C)
        out[b, 1024 * p:1024 * p + OUT_ROWS, :] = rows
    return out


# revision 24
# speedup vs baseline: 1.0276x; 1.0167x over previous
"""Trainium2 Bass kernel for EnhancedLocalAttentionWithGQA.

Problem (hardcoded): B=2, L=4096, C=2048, H=16 heads, D=128, G=2 kv groups,
window W=256 with stride 128 (50% overlap).

Key observation: the reference computes NW=31 overlapping windows but the
final output slice [:, :L] keeps only windows 0..15 (16 windows x 256 rows
= 4096 rows).  Window n's output rows [n*256,(n+1)*256) come from queries /
keys / values at input positions [n*128, n*128+256).  So only x positions
0..2175 feed QKV, and each window is an independent 256x256 attention.

Sharding (8 cores): core c -> batch b=c//4, quarter p=c%4, i.e. 4 windows
(global windows 4p..4p+3), input positions [512p, 512p+640), output rows
[1024p, 1024p+1024) of batch b.  No collectives; host concatenates rows.

Per-core pipeline (bf16 matmuls, fp32 PSUM), engineered so the PE never
waits:
  1. V-projection runs first, kc-outer, so it consumes x chunks as their
     DMAs land (x arrival is the startup bottleneck; lhsT=x chunks).
  2. K-projection, then Q per head (feat-major Q^T resident per head pair).
  3. Attention pairs (2 heads of one kv group, N=512 matmuls) interleaved
     1:1 with out-projection blocks of the PREVIOUS window, so softmax's
     ACT exp / DVE recip+mul hide entirely under out-proj matmul streams.
     Softmax sum uses an all-ones 128x128 lhsT so the colsum comes out of
     PSUM already broadcast across partitions (no ACT copy, no extra
     broadcast matmul).

All weights are host-pretiled so every DMA is a large contiguous transfer.
"""

import numpy as np
import ml_dtypes

import concourse.bacc as bacc
import concourse.tile as tile
from concourse import mybir
from concourse.bass_utils import run_bass_kernel_spmd

F32 = mybir.dt.float32
BF16 = mybir.dt.bfloat16

B = 2
L = 4096
C = 2048          # embed dim
H = 16            # heads
G = 2             # kv groups
D = 128           # head dim
KV = G * D        # 256
NWL = 4           # windows per core
S = NWL * 128 + 128   # 640 input positions per core
OUT_ROWS = NWL * 256  # 1024 output rows per core
KC = C // 128     # 16 contraction chunks
NT = 4            # out-proj 512-col tiles
SC_OUT = OUT_ROWS // 128
SCALE = 1.0 / float(np.sqrt(D))
N_CORES = 8

_CACHE = {}


def _build():
    nc = bacc.Bacc(None, target_bir_lowering=False)

    # host-pretiled layouts (see kernel() for the numpy side)
    xT_d = nc.dram_tensor("xT", [128, KC, S], BF16, kind="ExternalInput")
    wq_d = nc.dram_tensor("Wq", [H, 128, KC, 128], BF16, kind="ExternalInput")
    wk_d = nc.dram_tensor("Wk", [G, 128, KC, 128], BF16, kind="ExternalInput")
    wv_d = nc.dram_tensor("Wv", [128, KC, KV], BF16, kind="ExternalInput")
    wo_d = nc.dram_tensor("Wo", [NT, 128, KC, 512], BF16, kind="ExternalInput")
    bq_d = nc.dram_tensor("bq", [C], F32, kind="ExternalInput")
    bk_d = nc.dram_tensor("bk", [KV], F32, kind="ExternalInput")
    bv_d = nc.dram_tensor("bv", [KV], F32, kind="ExternalInput")
    bo_d = nc.dram_tensor("bo", [C], F32, kind="ExternalInput")
    out_d = nc.dram_tensor("out", [NT, SC_OUT, 128, 512], F32,
                           kind="ExternalOutput")

    with tile.TileContext(nc) as tc:
        with (
            tc.tile_pool(name="res", bufs=1) as res,
            tc.tile_pool(name="wqs", bufs=3) as wqs,
            tc.tile_pool(name="pts", bufs=3) as pts,
            tc.tile_pool(name="osb", bufs=3) as osbp,
            tc.tile_pool(name="norm", bufs=3) as norm,
        ):
            # ---------- resident loads (both queues, consumption order) ---
            # The early DMA rate is ~150GB/s per queue but queues add up,
            # so the startup-critical tensors (wv, x, wk, first wq) are
            # striped across the sync AND gpsimd queues: x even chunks on
            # sync, odd on gpsimd.  V-proj consumes x chunks as lhsT the
            # moment they land; then wk (K-proj), per-head wq, resident Wo.
            wv_t = res.tile([128, KC, KV], BF16, tag="wv", name="wv")
            xtq = [res.tile([128, 2, S], BF16, tag=f"xt{j}", name=f"xt{j}")
                   for j in range(8)]
            bq_sb = res.tile([128, H], F32, tag="bq", name="bq")
            nc.gpsimd.dma_start(out=bq_sb, in_=bq_d[:].rearrange("(h p) -> p h", p=128))
            bk_sb = res.tile([128, G], F32, tag="bk", name="bk")
            nc.gpsimd.dma_start(out=bk_sb, in_=bk_d[:].rearrange("(g p) -> p g", p=128))
            bv_bc = res.tile([128, KV], F32, tag="bvbc", name="bvbc")
            nc.gpsimd.dma_start(out=bv_bc,
                              in_=bv_d[:].unsqueeze(0).to_broadcast((128, KV)))

            kw = [res.tile([128, KC, 128], BF16, tag=f"kw{g}", name=f"kw{g}")
                  for g in range(G)]
            wq0s = [res.tile([128, 4, 128], BF16, tag=f"wq0{j}", name=f"wq0{j}")
                    for j in range(4)]
            # x is striped across both queues (the gpsimd SWDGE queue has
            # multi-us startup jitter, so it gets the later-consumed
            # chunks), wv rides between the sync x chunks.
            nc.sync.dma_start(out=wv_t[:, 0:4, :], in_=wv_d[:, 0:4, :])
            nc.sync.dma_start(out=xtq[0], in_=xT_d[:, 0:2, :])
            nc.gpsimd.dma_start(out=xtq[1], in_=xT_d[:, 2:4, :])
            nc.sync.dma_start(out=wv_t[:, 4:8, :], in_=wv_d[:, 4:8, :])
            nc.sync.dma_start(out=xtq[2], in_=xT_d[:, 4:6, :])
            nc.gpsimd.dma_start(out=xtq[3], in_=xT_d[:, 6:8, :])
            nc.sync.dma_start(out=wv_t[:, 8:12, :], in_=wv_d[:, 8:12, :])
            nc.sync.dma_start(out=xtq[4], in_=xT_d[:, 8:10, :])
            nc.gpsimd.dma_start(out=xtq[5], in_=xT_d[:, 10:12, :])
            nc.sync.dma_start(out=wv_t[:, 12:16, :], in_=wv_d[:, 12:16, :])
            nc.sync.dma_start(out=xtq[6], in_=xT_d[:, 12:14, :])
            nc.gpsimd.dma_start(out=xtq[7], in_=xT_d[:, 14:16, :])
            nc.gpsimd.dma_start(out=kw[0], in_=wk_d[0])
            nc.gpsimd.dma_start(out=kw[1], in_=wk_d[1])
            for j in range(4):
                nc.sync.dma_start(out=wq0s[j], in_=wq_d[0][:, j * 4:(j + 1) * 4, :])

            def xts(kc):
                return xtq[kc // 2][:, kc % 2, :]

            # needed only from the first out-proj block (~100us in)
            bo_bc = res.tile([128, C], F32, tag="bobc", name="bobc")
            nc.gpsimd.dma_start(out=bo_bc,
                              in_=bo_d[:].unsqueeze(0).to_broadcast((128, C)))

            ones128 = res.tile([128, 128], BF16, tag="ones", name="ones")
            nc.vector.memset(ones128, 1.0)
            zcol = res.tile([128, 1], F32, tag="zcol", name="zcol")
            nc.vector.memset(zcol, 0.0)
            # dummy exp: preload the ACT exp table set while the PE is
            # still waiting on the x DMA (the real first exp would other-
            # wise eat the ~2.7us table load mid-pipeline).
            dummy = res.tile([128, 1], F32, tag="dummy", name="dummy")
            nc.scalar.activation(dummy, zcol,
                                 mybir.ActivationFunctionType.Exp)

            # paired Q storage: qp[g*4+j] holds heads (g+4j, g+4j+2)
            qp = [res.tile([128, 2, S], BF16, tag=f"qp{i}", name=f"qp{i}")
                  for i in range(8)]

            def q_slot(h):
                g, k = h % G, h // G
                return qp[g * 4 + k // 2][:, k % 2, :]
            kt = [res.tile([128, S], BF16, tag=f"kt{g}", name=f"kt{g}")
                  for g in range(G)]
            vt = [res.tile([128, KV], BF16, tag=f"vt{sc}", name=f"vt{sc}")
                  for sc in range(S // 128)]
            # O^T per window (separate tiles so out-proj reads of window
            # w-1 never alias the concurrent writes of window w)
            otw = [res.tile([128, H, 256], BF16, tag=f"ot{w}", name=f"ot{w}")
                   for w in range(NWL)]

            NA, NB = 320, 320  # free split of S=640 (psum bank = 512 f32)

            # ---------- V projection ----------
            with tc.tile_pool(name="psA", bufs=1, space="PSUM") as psA:
                # HAM warm-up: the PE clock-gate defaults to 4/8 (1.2GHz)
                # and needs ~3.4us of sustained matmul activity to open.
                # The PE idles waiting on the x DMA anyway, so burn that
                # window on dummy matmuls and enter V/K projections warm.
                warm = psA.tile([128, 128], F32, tag="warm", name="warm")
                for _ in range(64):
                    nc.tensor.matmul(warm, lhsT=ones128, rhs=ones128,
                                     start=True, stop=True)
                # V + head-0 Q, kc-outer: each x chunk enables matmuls the
                # moment it lands, so the PE tracks the x DMA instead of
                # idling behind it.  Accumulation order follows expected
                # DMA arrival (sync evens lead; the gpsimd SWDGE queue
                # starts ~6us later).  PSUM: warm 1 + pv 5 + pa/pb 2 = 8.
                pvs = [psA.tile([128, KV], F32, tag=f"vp{sc}", name=f"vp{sc}")
                       for sc in range(S // 128)]
                vorder = [0, 1, 4, 5, 2, 3, 8, 9, 6, 7, 12, 13, 10, 11, 14, 15]
                for idx, kc in enumerate(vorder):
                    for sc in range(S // 128):
                        nc.tensor.matmul(
                            pvs[sc], lhsT=xts(kc)[:, sc * 128:(sc + 1) * 128],
                            rhs=wv_t[:, kc, :],
                            start=(idx == 0), stop=(idx == KC - 1))
                for sc in range(S // 128):
                    nc.vector.tensor_add(vt[sc], pvs[sc], bv_bc)

            # resident Wo, queued behind the wq head stream on purpose:
            # needed only from the first out-proj block (~100us in).
            wo_all = [res.tile([128, KC, 512], BF16, tag=f"wo{nt}",
                               name=f"wo{nt}") for nt in range(NT)]

            # ---------- K/Q projections + attention + out-projection ------
            # pair i = (window w, kv group g, head pair j): heads g+4j and
            # g+4j+2, query cols w*128..w*128+256.  Window 0's pairs are
            # injected into the Q-head stream (their ACT/DVE work hides
            # under the projection matmuls); pairs of windows 1..3 are
            # interleaved 1:1 with out-proj blocks of window w-1, so the
            # softmax's ACT/DVE work never gates the PE.
            pairs = [(w, g, j) for w in range(NWL)
                     for g in range(G) for j in range(4)]
            NP = len(pairs)
            pt_tiles = {}

            with (
                tc.tile_pool(name="psB", bufs=1, space="PSUM") as psB,
                tc.tile_pool(name="psS", bufs=1, space="PSUM") as psS,
                tc.tile_pool(name="psO", bufs=1, space="PSUM") as psO,
                tc.tile_pool(name="psC", bufs=1, space="PSUM") as psC,
                tc.tile_pool(name="psP", bufs=2, space="PSUM") as psP,
            ):
                korder = [0, 1, 4, 5, 2, 3, 8, 9, 6, 7, 12, 13, 10, 11, 14, 15]
                for g in range(G):
                    pa = psB.tile([128, NA], F32, tag="qa", name="qa")
                    pb = psB.tile([128, NB], F32, tag="qb", name="qb")
                    for idx, kc in enumerate(korder):
                        nc.tensor.matmul(pa, lhsT=kw[g][:, kc, :],
                                         rhs=xts(kc)[:, 0:NA],
                                         start=(idx == 0), stop=(idx == KC - 1))
                    for kc in range(KC):
                        nc.tensor.matmul(pb, lhsT=kw[g][:, kc, :],
                                         rhs=xts(kc)[:, NA:S],
                                         start=(kc == 0), stop=(kc == KC - 1))
                    nc.scalar.activation(kt[g][:, 0:NA], pa,
                                         mybir.ActivationFunctionType.Identity,
                                         bias=bk_sb[:, g:g + 1])
                    nc.scalar.activation(kt[g][:, NA:S], pb,
                                         mybir.ActivationFunctionType.Identity,
                                         bias=bk_sb[:, g:g + 1])
                def pair_qk(i):
                    w, g, j = pairs[i]
                    q0 = w * 128
                    qpt = qp[g * 4 + j]
                    st = psS.tile([128, 2, 512], F32, tag="st", name="st")
                    for kc in range(2):
                        nc.tensor.matmul(
                            st[:, kc, :],
                            lhsT=kt[g][:, q0 + kc * 128:q0 + (kc + 1) * 128],
                            rhs=qpt[:, :, q0:q0 + 256],
                            start=True, stop=True)
                    pt = pts.tile([128, 2, 512], BF16, tag="pt", name="pt")
                    nc.scalar.activation(pt, st,
                                         mybir.ActivationFunctionType.Exp,
                                         scale=SCALE)
                    pt_tiles[i] = pt

                def pair_av(i):
                    w, g, j = pairs[i]
                    h0, h1 = g + 4 * j, g + 4 * j + 2
                    pt = pt_tiles.pop(i)
                    ob = psO.tile([128, 512], F32, tag="ob", name="ob")
                    for kc in range(2):
                        nc.tensor.matmul(ob,
                                         lhsT=vt[w + kc][:, g * 128:(g + 1) * 128],
                                         rhs=pt[:, kc, :],
                                         start=(kc == 0), stop=(kc == 1))
                    # all-ones lhsT: every PSUM partition gets the colsum,
                    # i.e. the softmax denominator arrives pre-broadcast.
                    bc = psC.tile([128, 512], F32, tag="bc", name="bc")
                    for kc in range(2):
                        nc.tensor.matmul(bc, lhsT=ones128, rhs=pt[:, kc, :],
                                         start=(kc == 0), stop=(kc == 1))
                    bcr = norm.tile([128, 512], F32, tag="bcr", name="bcr")
                    nc.vector.reciprocal_approx_fast(out=bcr, in_=bc)
                    nc.vector.tensor_mul(otw[w][:, h0, :], ob[:, 0:256],
                                         bcr[:, 0:256])
                    nc.vector.tensor_mul(otw[w][:, h1, :], ob[:, 256:512],
                                         bcr[:, 256:512])

                def outproj_block(w_src, b, last=False):
                    nt, half = b % NT, b // NT
                    sc = 2 * w_src + half
                    po = psP.tile([128, 512], F32, tag="op", name="op")
                    for fc in range(KC):
                        nc.tensor.matmul(
                            po,
                            lhsT=otw[w_src][:, fc, half * 128:(half + 1) * 128],
                            rhs=wo_all[nt][:, fc, :],
                            start=(fc == 0), stop=(fc == KC - 1))
                    ob_sb = osbp.tile([128, 512], F32, tag="osb", name="osb")
                    if last:
                        # split the final block so its DVE add and DMA
                        # pipeline instead of serializing the kernel tail
                        for hx in range(4):
                            cs = slice(hx * 128, (hx + 1) * 128)
                            nc.vector.tensor_add(ob_sb[:, cs], po[:, cs],
                                                 bo_bc[:, nt * 512 + hx * 128:
                                                       nt * 512 + (hx + 1) * 128])
                            nc.sync.dma_start(
                                out=out_d[nt, sc, :, cs], in_=ob_sb[:, cs])
                    else:
                        nc.vector.tensor_add(ob_sb, po,
                                             bo_bc[:, nt * 512:(nt + 1) * 512])
                        nc.sync.dma_start(out=out_d[nt, sc], in_=ob_sb)

                def q_head(h):
                    if h > 0:
                        wq_t = wqs.tile([128, KC, 128], BF16, tag="wq", name="wq")
                        nc.sync.dma_start(out=wq_t, in_=wq_d[h])

                    def wql(kc, h=h, wq_t=(None if h == 0 else wq_t)):
                        if h == 0:
                            return wq0s[kc // 4][:, kc % 4, :]
                        return wq_t[:, kc, :]
                    pa = psB.tile([128, NA], F32, tag="qa", name="qa")
                    pb = psB.tile([128, NB], F32, tag="qb", name="qb")
                    for kc in range(KC):
                        nc.tensor.matmul(pa, lhsT=wql(kc),
                                         rhs=xts(kc)[:, 0:NA],
                                         start=(kc == 0), stop=(kc == KC - 1))
                    for kc in range(KC):
                        nc.tensor.matmul(pb, lhsT=wql(kc),
                                         rhs=xts(kc)[:, NA:S],
                                         start=(kc == 0), stop=(kc == KC - 1))
                    nc.scalar.activation(q_slot(h)[:, 0:NA], pa,
                                         mybir.ActivationFunctionType.Identity,
                                         bias=bq_sb[:, h:h + 1])
                    nc.scalar.activation(q_slot(h)[:, NA:S], pb,
                                         mybir.ActivationFunctionType.Identity,
                                         bias=bq_sb[:, h:h + 1])

                # Q heads in pair order, window-0 pair p injected as soon
                # as its two heads exist (pairs[p] = (0, p//4, p%4) covers
                # heads p//4 + 4*(p%4) and +2, i.e. exactly one hp pair).
                for p in range(8):
                    g, j = p // 4, p % 4
                    q_head(g + 4 * j)
                    q_head(g + 4 * j + 2)
                    pair_qk(p)
                    if p > 0:
                        pair_av(p - 1)
                pair_av(7)
                for nt in range(NT):
                    nc.sync.dma_start(out=wo_all[nt], in_=wo_d[nt])

                pair_qk(8)
                for i in range(8, NP):
                    pair_av(i)
                    if i + 1 < NP:
                        pair_qk(i + 1)
                    outproj_block(pairs[i][0] - 1, i % 8)
                for b in range(8):
                    outproj_block(NWL - 1, b, last=(b == 7))

    nc.compile()
    return nc


def _get_nc():
    if "nc" not in _CACHE:
        _CACHE["nc"] = _build()
    return _CACHE["nc"]


def _prep_weights(Wq, bq, Wk, bk, Wv, bv, Wo, bo):
    bf16 = ml_dtypes.bfloat16
    f32 = lambda a: np.ascontiguousarray(np.asarray(a, dtype=np.float32))
    wq = np.asarray(Wq, np.float32).reshape(KC, 128, H, 128)
    wq = np.ascontiguousarray(wq.transpose(2, 1, 0, 3)).astype(bf16)  # (H,p,kc,f)
    wk = np.asarray(Wk, np.float32).reshape(KC, 128, G, 128)
    wk = np.ascontiguousarray(wk.transpose(2, 1, 0, 3)).astype(bf16)  # (G,p,kc,f)
    wv = np.asarray(Wv, np.float32).reshape(KC, 128, KV)
    wv = np.ascontiguousarray(wv.transpose(1, 0, 2)).astype(bf16)     # (p,kc,f)
    wo = np.asarray(Wo, np.float32).reshape(KC, 128, NT, 512)
    wo = np.ascontiguousarray(wo.transpose(2, 1, 0, 3)).astype(bf16)  # (NT,p,kc,f)
    return {
        "Wq": wq, "Wk": wk, "Wv": wv, "Wo": wo,
        "bq": f32(bq), "bk": f32(bk), "bv": f32(bv), "bo": f32(bo),
    }


def _prep_in_maps(x, weights):
    bf16 = ml_dtypes.bfloat16
    in_maps = []
    for c in range(N_CORES):
        b, p = divmod(c, 4)
        xs = np.asarray(x[b, 512 * p:512 * p + S, :], np.float32)
        xT = np.ascontiguousarray(xs.T.reshape(KC, 128, S).transpose(1, 0, 2))
        in_maps.append(dict(weights, xT=xT.astype(bf16)))
    return in_maps


def kernel(x, Wq, bq, Wk, bk, Wv, bv, Wo, bo, **_):
    x = np.asarray(x, dtype=np.float32)
    weights = _prep_weights(Wq, bq, Wk, bk, Wv, bv, Wo, bo)
    in_maps = _prep_in_maps(x, weights)

    nc = _get_nc()
    res = run_bass_kernel_spmd(nc, in_maps, core_ids=list(range(N_CORES)))

    out = np.empty((B, L, C), dtype=np.float32)
    for c in range(N_CORES):
        b, p = divmod(c, 4)
        blk = res.results[c]["out"]  # (NT, SC_OUT, 128, 512)
        rows = blk.transpose(1, 2, 0, 3).reshape(OUT_ROWS, C)
        out[b, 1024 * p:1024 * p + OUT_ROWS, :] = rows
    return out


# revision 26
# speedup vs baseline: 1.0469x; 1.0188x over previous
"""Trainium2 Bass kernel for EnhancedLocalAttentionWithGQA.

Problem (hardcoded): B=2, L=4096, C=2048, H=16 heads, D=128, G=2 kv groups,
window W=256 with stride 128 (50% overlap).

Key observation: the reference computes NW=31 overlapping windows but the
final output slice [:, :L] keeps only windows 0..15 (16 windows x 256 rows
= 4096 rows).  Window n's output rows [n*256,(n+1)*256) come from queries /
keys / values at input positions [n*128, n*128+256).  So only x positions
0..2175 feed QKV, and each window is an independent 256x256 attention.

Sharding (8 cores): core c -> batch b=c//4, quarter p=c%4, i.e. 4 windows
(global windows 4p..4p+3), input positions [512p, 512p+640), output rows
[1024p, 1024p+1024) of batch b.  No collectives; host concatenates rows.

Per-core pipeline (bf16 matmuls, fp32 PSUM), engineered so the PE never
waits:
  1. V-projection runs first, kc-outer, so it consumes x chunks as their
     DMAs land (x arrival is the startup bottleneck; lhsT=x chunks).
  2. K-projection, then Q per head (feat-major Q^T resident per head pair).
  3. Attention pairs (2 heads of one kv group, N=512 matmuls) interleaved
     1:1 with out-projection blocks of the PREVIOUS window, so softmax's
     ACT exp / DVE recip+mul hide entirely under out-proj matmul streams.
     Softmax sum uses an all-ones 128x128 lhsT so the colsum comes out of
     PSUM already broadcast across partitions (no ACT copy, no extra
     broadcast matmul).

All weights are host-pretiled so every DMA is a large contiguous transfer.
"""

import numpy as np
import ml_dtypes

import concourse.bacc as bacc
import concourse.tile as tile
from concourse import mybir
from concourse.bass_utils import run_bass_kernel_spmd

F32 = mybir.dt.float32
BF16 = mybir.dt.bfloat16

B = 2
L = 4096
C = 2048          # embed dim
H = 16            # heads
G = 2             # kv groups
D = 128           # head dim
KV = G * D        # 256
NWL = 4           # windows per core
S = NWL * 128 + 128   # 640 input positions per core
OUT_ROWS = NWL * 256  # 1024 output rows per core
KC = C // 128     # 16 contraction chunks
NT = 4            # out-proj 512-col tiles
SC_OUT = OUT_ROWS // 128
SCALE = 1.0 / float(np.sqrt(D))
N_CORES = 8

_CACHE = {}


def _build():
    nc = bacc.Bacc(None, target_bir_lowering=False)

    # host-pretiled layouts (see kernel() for the numpy side)
    xT_d = nc.dram_tensor("xT", [128, KC, S], BF16, kind="ExternalInput")
    wq_d = nc.dram_tensor("Wq", [H, 128, KC, 128], BF16, kind="ExternalInput")
    wk_d = nc.dram_tensor("Wk", [G, 128, KC, 128], BF16, kind="ExternalInput")
    wv_d = nc.dram_tensor("Wv", [128, KC, KV], BF16, kind="ExternalInput")
    wo_d = nc.dram_tensor("Wo", [NT, 128, KC, 512], BF16, kind="ExternalInput")
    bq_d = nc.dram_tensor("bq", [C], F32, kind="ExternalInput")
    bk_d = nc.dram_tensor("bk", [KV], F32, kind="ExternalInput")
    bv_d = nc.dram_tensor("bv", [KV], F32, kind="ExternalInput")
    bo_d = nc.dram_tensor("bo", [C], F32, kind="ExternalInput")
    out_d = nc.dram_tensor("out", [NT, SC_OUT, 128, 512], F32,
                           kind="ExternalOutput")

    with tile.TileContext(nc) as tc:
        with (
            tc.tile_pool(name="res", bufs=1) as res,
            tc.tile_pool(name="wqs", bufs=3) as wqs,
            tc.tile_pool(name="pts", bufs=3) as pts,
            tc.tile_pool(name="osb", bufs=3) as osbp,
            tc.tile_pool(name="norm", bufs=3) as norm,
        ):
            # ---------- resident loads (both queues, consumption order) ---
            # The early DMA rate is ~150GB/s per queue but queues add up,
            # so the startup-critical tensors (wv, x, wk, first wq) are
            # striped across the sync AND gpsimd queues: x even chunks on
            # sync, odd on gpsimd.  V-proj consumes x chunks as lhsT the
            # moment they land; then wk (K-proj), per-head wq, resident Wo.
            wv_t = res.tile([128, KC, KV], BF16, tag="wv", name="wv")
            xtq = [res.tile([128, 2, S], BF16, tag=f"xt{j}", name=f"xt{j}")
                   for j in range(8)]
            bq_sb = res.tile([128, H], F32, tag="bq", name="bq")
            nc.gpsimd.dma_start(out=bq_sb, in_=bq_d[:].rearrange("(h p) -> p h", p=128))
            bk_sb = res.tile([128, G], F32, tag="bk", name="bk")
            nc.gpsimd.dma_start(out=bk_sb, in_=bk_d[:].rearrange("(g p) -> p g", p=128))
            bv_bc = res.tile([128, KV], F32, tag="bvbc", name="bvbc")
            nc.gpsimd.dma_start(out=bv_bc,
                              in_=bv_d[:].unsqueeze(0).to_broadcast((128, KV)))

            kw = [res.tile([128, KC, 128], BF16, tag=f"kw{g}", name=f"kw{g}")
                  for g in range(G)]
            wq0s = [res.tile([128, 4, 128], BF16, tag=f"wq0{j}", name=f"wq0{j}")
                    for j in range(4)]
            # x is striped across both queues (the gpsimd SWDGE queue has
            # multi-us startup jitter, so it gets the later-consumed
            # chunks), wv rides between the sync x chunks.
            nc.sync.dma_start(out=wv_t[:, 0:4, :], in_=wv_d[:, 0:4, :])
            nc.sync.dma_start(out=xtq[0], in_=xT_d[:, 0:2, :])
            nc.gpsimd.dma_start(out=xtq[1], in_=xT_d[:, 2:4, :])
            nc.sync.dma_start(out=wv_t[:, 4:8, :], in_=wv_d[:, 4:8, :])
            nc.sync.dma_start(out=xtq[2], in_=xT_d[:, 4:6, :])
            nc.gpsimd.dma_start(out=xtq[3], in_=xT_d[:, 6:8, :])
            nc.sync.dma_start(out=wv_t[:, 8:12, :], in_=wv_d[:, 8:12, :])
            nc.sync.dma_start(out=xtq[4], in_=xT_d[:, 8:10, :])
            nc.gpsimd.dma_start(out=xtq[5], in_=xT_d[:, 10:12, :])
            nc.sync.dma_start(out=wv_t[:, 12:16, :], in_=wv_d[:, 12:16, :])
            nc.sync.dma_start(out=xtq[6], in_=xT_d[:, 12:14, :])
            nc.gpsimd.dma_start(out=xtq[7], in_=xT_d[:, 14:16, :])
            nc.gpsimd.dma_start(out=kw[0], in_=wk_d[0])
            nc.gpsimd.dma_start(out=kw[1], in_=wk_d[1])
            for j in range(4):
                nc.sync.dma_start(out=wq0s[j], in_=wq_d[0][:, j * 4:(j + 1) * 4, :])

            def xts(kc):
                return xtq[kc // 2][:, kc % 2, :]

            # needed only from the first out-proj block (~100us in)
            bo_bc = res.tile([128, C], F32, tag="bobc", name="bobc")
            nc.gpsimd.dma_start(out=bo_bc,
                              in_=bo_d[:].unsqueeze(0).to_broadcast((128, C)))

            ones128 = res.tile([128, 128], BF16, tag="ones", name="ones")
            nc.vector.memset(ones128, 1.0)
            zcol = res.tile([128, 1], F32, tag="zcol", name="zcol")
            nc.vector.memset(zcol, 0.0)
            # dummy exp: preload the ACT exp table set while the PE is
            # still waiting on the x DMA (the real first exp would other-
            # wise eat the ~2.7us table load mid-pipeline).
            dummy = res.tile([128, 1], F32, tag="dummy", name="dummy")
            nc.scalar.activation(dummy, zcol,
                                 mybir.ActivationFunctionType.Exp)

            # paired Q storage: qp[g*4+j] holds heads (g+4j, g+4j+2)
            qp = [res.tile([128, 2, S], BF16, tag=f"qp{i}", name=f"qp{i}")
                  for i in range(8)]

            def q_slot(h):
                g, k = h % G, h // G
                return qp[g * 4 + k // 2][:, k % 2, :]
            kt = [res.tile([128, S], BF16, tag=f"kt{g}", name=f"kt{g}")
                  for g in range(G)]
            vt = [res.tile([128, KV], BF16, tag=f"vt{sc}", name=f"vt{sc}")
                  for sc in range(S // 128)]
            # O^T per window (separate tiles so out-proj reads of window
            # w-1 never alias the concurrent writes of window w)
            otw = [res.tile([128, H, 256], BF16, tag=f"ot{w}", name=f"ot{w}")
                   for w in range(NWL)]

            NA, NB = 320, 320  # free split of S=640 (psum bank = 512 f32)

            # ---------- V projection ----------
            with tc.tile_pool(name="psA", bufs=1, space="PSUM") as psA:
                # HAM warm-up: the PE clock-gate defaults to 4/8 (1.2GHz)
                # and needs ~3.4us of sustained matmul activity to open.
                # The PE idles waiting on the x DMA anyway, so burn that
                # window on dummy matmuls and enter V/K projections warm.
                warm = psA.tile([128, 128], F32, tag="warm", name="warm")
                for _ in range(64):
                    nc.tensor.matmul(warm, lhsT=ones128, rhs=ones128,
                                     start=True, stop=True)
                # V + head-0 Q, kc-outer: each x chunk enables matmuls the
                # moment it lands, so the PE tracks the x DMA instead of
                # idling behind it.  Accumulation order follows expected
                # DMA arrival (sync evens lead; the gpsimd SWDGE queue
                # starts ~6us later).  PSUM: warm 1 + pv 5 + pa/pb 2 = 8.
                pvs = [psA.tile([128, KV], F32, tag=f"vp{sc}", name=f"vp{sc}")
                       for sc in range(S // 128)]
                vorder = [0, 1, 4, 5, 2, 3, 8, 9, 6, 7, 12, 13, 10, 11, 14, 15]
                for idx, kc in enumerate(vorder):
                    for sc in range(S // 128):
                        nc.tensor.matmul(
                            pvs[sc], lhsT=xts(kc)[:, sc * 128:(sc + 1) * 128],
                            rhs=wv_t[:, kc, :],
                            start=(idx == 0), stop=(idx == KC - 1))
                for sc in range(S // 128):
                    nc.vector.tensor_add(vt[sc], pvs[sc], bv_bc)

            # resident Wo, queued behind the wq head stream on purpose:
            # needed only from the first out-proj block (~100us in).
            wo_all = [res.tile([128, KC, 512], BF16, tag=f"wo{nt}",
                               name=f"wo{nt}") for nt in range(NT)]

            # ---------- K/Q projections + attention + out-projection ------
            # pair i = (window w, kv group g, head pair j): heads g+4j and
            # g+4j+2, query cols w*128..w*128+256.  Window 0's pairs are
            # injected into the Q-head stream (their ACT/DVE work hides
            # under the projection matmuls); pairs of windows 1..3 are
            # interleaved 1:1 with out-proj blocks of window w-1, so the
            # softmax's ACT/DVE work never gates the PE.
            pairs = [(w, g, j) for w in range(NWL)
                     for g in range(G) for j in range(4)]
            NP = len(pairs)
            pt_tiles = {}

            with (
                tc.tile_pool(name="psB", bufs=1, space="PSUM") as psB,
                tc.tile_pool(name="psS", bufs=1, space="PSUM") as psS,
                tc.tile_pool(name="psO", bufs=1, space="PSUM") as psO,
                tc.tile_pool(name="psC", bufs=1, space="PSUM") as psC,
                tc.tile_pool(name="psP", bufs=2, space="PSUM") as psP,
            ):
                korder = [0, 1, 4, 5, 2, 3, 8, 9, 6, 7, 12, 13, 10, 11, 14, 15]
                for g in range(G):
                    pa = psB.tile([128, NA], F32, tag="qa", name="qa")
                    pb = psB.tile([128, NB], F32, tag="qb", name="qb")
                    for idx, kc in enumerate(korder):
                        nc.tensor.matmul(pa, lhsT=kw[g][:, kc, :],
                                         rhs=xts(kc)[:, 0:NA],
                                         start=(idx == 0), stop=(idx == KC - 1))
                    for kc in range(KC):
                        nc.tensor.matmul(pb, lhsT=kw[g][:, kc, :],
                                         rhs=xts(kc)[:, NA:S],
                                         start=(kc == 0), stop=(kc == KC - 1))
                    nc.scalar.activation(kt[g][:, 0:NA], pa,
                                         mybir.ActivationFunctionType.Identity,
                                         bias=bk_sb[:, g:g + 1])
                    nc.scalar.activation(kt[g][:, NA:S], pb,
                                         mybir.ActivationFunctionType.Identity,
                                         bias=bk_sb[:, g:g + 1])
                def pair_qk(i):
                    w, g, j = pairs[i]
                    q0 = w * 128
                    qpt = qp[g * 4 + j]
                    st = psS.tile([128, 2, 512], F32, tag="st", name="st")
                    for kc in range(2):
                        nc.tensor.matmul(
                            st[:, kc, :],
                            lhsT=kt[g][:, q0 + kc * 128:q0 + (kc + 1) * 128],
                            rhs=qpt[:, :, q0:q0 + 256],
                            start=True, stop=True)
                    pt = pts.tile([128, 2, 512], BF16, tag="pt", name="pt")
                    nc.scalar.activation(pt, st,
                                         mybir.ActivationFunctionType.Exp,
                                         scale=SCALE)
                    # pre-sum the two key chunks on the (otherwise idle)
                    # DVE so the colsum-broadcast needs one matmul, not two
                    s2 = pts.tile([128, 512], BF16, tag="s2", name="s2")
                    nc.vector.tensor_add(s2, pt[:, 0, :], pt[:, 1, :])
                    pt_tiles[i] = (pt, s2)

                def pair_av(i):
                    w, g, j = pairs[i]
                    h0, h1 = g + 4 * j, g + 4 * j + 2
                    pt, s2 = pt_tiles.pop(i)
                    ob = psO.tile([128, 512], F32, tag="ob", name="ob")
                    for kc in range(2):
                        nc.tensor.matmul(ob,
                                         lhsT=vt[w + kc][:, g * 128:(g + 1) * 128],
                                         rhs=pt[:, kc, :],
                                         start=(kc == 0), stop=(kc == 1))
                    # all-ones lhsT: every PSUM partition gets the colsum,
                    # i.e. the softmax denominator arrives pre-broadcast.
                    bc = psC.tile([128, 512], F32, tag="bc", name="bc")
                    nc.tensor.matmul(bc, lhsT=ones128, rhs=s2,
                                     start=True, stop=True)
                    bcr = norm.tile([128, 512], F32, tag="bcr", name="bcr")
                    nc.vector.reciprocal_approx_fast(out=bcr, in_=bc)
                    nc.vector.tensor_mul(otw[w][:, h0, :], ob[:, 0:256],
                                         bcr[:, 0:256])
                    nc.vector.tensor_mul(otw[w][:, h1, :], ob[:, 256:512],
                                         bcr[:, 256:512])

                def outproj_block(w_src, b, last=False):
                    nt, half = b % NT, b // NT
                    sc = 2 * w_src + half
                    po = psP.tile([128, 512], F32, tag="op", name="op")
                    for fc in range(KC):
                        nc.tensor.matmul(
                            po,
                            lhsT=otw[w_src][:, fc, half * 128:(half + 1) * 128],
                            rhs=wo_all[nt][:, fc, :],
                            start=(fc == 0), stop=(fc == KC - 1))
                    ob_sb = osbp.tile([128, 512], F32, tag="osb", name="osb")
                    if last:
                        # split the final block so its DVE add and DMA
                        # pipeline instead of serializing the kernel tail
                        for hx in range(4):
                            cs = slice(hx * 128, (hx + 1) * 128)
                            nc.vector.tensor_add(ob_sb[:, cs], po[:, cs],
                                                 bo_bc[:, nt * 512 + hx * 128:
                                                       nt * 512 + (hx + 1) * 128])
                            nc.sync.dma_start(
                                out=out_d[nt, sc, :, cs], in_=ob_sb[:, cs])
                    else:
                        nc.vector.tensor_add(ob_sb, po,
                                             bo_bc[:, nt * 512:(nt + 1) * 512])
                        nc.sync.dma_start(out=out_d[nt, sc], in_=ob_sb)

                def q_head(h):
                    if h > 0:
                        wq_t = wqs.tile([128, KC, 128], BF16, tag="wq", name="wq")
                        nc.sync.dma_start(out=wq_t, in_=wq_d[h])

                    def wql(kc, h=h, wq_t=(None if h == 0 else wq_t)):
                        if h == 0:
                            return wq0s[kc // 4][:, kc % 4, :]
                        return wq_t[:, kc, :]
                    pa = psB.tile([128, NA], F32, tag="qa", name="qa")
                    pb = psB.tile([128, NB], F32, tag="qb", name="qb")
                    for kc in range(KC):
                        nc.tensor.matmul(pa, lhsT=wql(kc),
                                         rhs=xts(kc)[:, 0:NA],
                                         start=(kc == 0), stop=(kc == KC - 1))
                    for kc in range(KC):
                        nc.tensor.matmul(pb, lhsT=wql(kc),
                                         rhs=xts(kc)[:, NA:S],
                                         start=(kc == 0), stop=(kc == KC - 1))
                    nc.scalar.activation(q_slot(h)[:, 0:NA], pa,
                                         mybir.ActivationFunctionType.Identity,
                                         bias=bq_sb[:, h:h + 1])
                    nc.scalar.activation(q_slot(h)[:, NA:S], pb,
                                         mybir.ActivationFunctionType.Identity,
                                         bias=bq_sb[:, h:h + 1])

                # Q heads in pair order, window-0 pair p injected as soon
                # as its two heads exist (pairs[p] = (0, p//4, p%4) covers
                # heads p//4 + 4*(p%4) and +2, i.e. exactly one hp pair).
                for p in range(8):
                    g, j = p // 4, p % 4
                    q_head(g + 4 * j)
                    q_head(g + 4 * j + 2)
                    pair_qk(p)
                    if p > 0:
                        pair_av(p - 1)
                pair_av(7)
                for nt in range(NT):
                    nc.sync.dma_start(out=wo_all[nt], in_=wo_d[nt])

                pair_qk(8)
                for i in range(8, NP):
                    pair_av(i)
                    if i + 1 < NP:
                        pair_qk(i + 1)
                    outproj_block(pairs[i][0] - 1, i % 8)
                for b in range(8):
                    outproj_block(NWL - 1, b, last=(b == 7))

    nc.compile()
    return nc


def _get_nc():
    if "nc" not in _CACHE:
        _CACHE["nc"] = _build()
    return _CACHE["nc"]


def _prep_weights(Wq, bq, Wk, bk, Wv, bv, Wo, bo):
    bf16 = ml_dtypes.bfloat16
    f32 = lambda a: np.ascontiguousarray(np.asarray(a, dtype=np.float32))
    wq = np.asarray(Wq, np.float32).reshape(KC, 128, H, 128)
    wq = np.ascontiguousarray(wq.transpose(2, 1, 0, 3)).astype(bf16)  # (H,p,kc,f)
    wk = np.asarray(Wk, np.float32).reshape(KC, 128, G, 128)
    wk = np.ascontiguousarray(wk.transpose(2, 1, 0, 3)).astype(bf16)  # (G,p,kc,f)
    wv = np.asarray(Wv, np.float32).reshape(KC, 128, KV)
    wv = np.ascontiguousarray(wv.transpose(1, 0, 2)).astype(bf16)     # (p,kc,f)
    wo = np.asarray(Wo, np.float32).reshape(KC, 128, NT, 512)
    wo = np.ascontiguousarray(wo.transpose(2, 1, 0, 3)).astype(bf16)  # (NT,p,kc,f)
    return {
        "Wq": wq, "Wk": wk, "Wv": wv, "Wo": wo,
        "bq": f32(bq), "bk": f32(bk), "bv": f32(bv), "bo": f32(bo),
    }


def _prep_in_maps(x, weights):
    bf16 = ml_dtypes.bfloat16
    in_maps = []
    for c in range(N_CORES):
        b, p = divmod(c, 4)
        xs = np.asarray(x[b, 512 * p:512 * p + S, :], np.float32)
        xT = np.ascontiguousarray(xs.T.reshape(KC, 128, S).transpose(1, 0, 2))
        in_maps.append(dict(weights, xT=xT.astype(bf16)))
    return in_maps


def kernel(x, Wq, bq, Wk, bk, Wv, bv, Wo, bo, **_):
    x = np.asarray(x, dtype=np.float32)
    weights = _prep_weights(Wq, bq, Wk, bk, Wv, bv, Wo, bo)
    in_maps = _prep_in_maps(x, weights)

    nc = _get_nc()
    res = run_bass_kernel_spmd(nc, in_maps, core_ids=list(range(N_CORES)))

    out = np.empty((B, L, C), dtype=np.float32)
    for c in range(N_CORES):
        b, p = divmod(c, 4)
        blk = res.results[c]["out"]  # (NT, SC_OUT, 128, 512)
        rows = blk.transpose(1, 2, 0, 3).reshape(OUT_ROWS, C)
        out[b, 1024 * p:1024 * p + OUT_ROWS, :] = rows
    return out
